# revision 13
# baseline (speedup 1.0000x reference)
# Trainium2 Bass kernel for NER CRF forward (loss + viterbi predictions).
# Shards batch (64 seqs) across 8 cores (8 seqs each). Chunked-scan CRF:
#   T=512 -> 16 chunks x 32 steps; matrix-state transfer scans, exp-space
#   NLL combine (PE blockdiag matmuls), max-plus viterbi + E-chain backtrace.
import sys
sys.path.insert(0, '/opt/trn_rl_repo')
import numpy as np
import ml_dtypes
import concourse.bass as bass
import concourse.bacc as bacc
import concourse.mybir as mybir
from concourse import tile
from concourse.bass_utils import run_bass_kernel_spmd

F32 = mybir.dt.float32
BF16 = mybir.dt.bfloat16
I32 = mybir.dt.int32
AX = mybir.AxisListType
OP = mybir.AluOpType
ACT = mybir.ActivationFunctionType

B, T, HD, L = 64, 512, 768, 9
NC = 8          # cores
BL = 8          # seqs per core
C, S = 16, 32   # chunks x steps; row = b*16 + c
KAPPA = 2.34
LNSCALE = 2.0 ** -32
KH = 6          # h tiles of 128

def V(t, dims, off=0):
    return bass.AP(t.tensor, t.offset + off, [list(t.ap[0])] + [list(d) for d in dims])

_prog_cache = {}

def build_program(debug=False):
    if ('p', debug) in _prog_cache:
        return _prog_cache[('p', debug)]
    nc = bacc.Bacc("TRN2", target_bir_lowering=False, debug=False, num_devices=NC)
    dt = nc.dram_tensor
    # ---- inputs (per core) ----
    hhi = dt("hhi", [BL, T, HD], BF16, kind="ExternalInput")
    hlo = dt("hlo", [BL, T, HD], BF16, kind="ExternalInput")
    wrep_hi = dt("wrep_hi", [KH, 128, 81], BF16, kind="ExternalInput")
    wrep_lo = dt("wrep_lo", [KH, 128, 81], BF16, kind="ExternalInput")
    eblk_d = dt("eblk_d", [81, 81], F32, kind="ExternalInput")      # blockdiag exp(TR)
    t81_d  = dt("t81_d", [128, 81], F32, kind="ExternalInput")      # (j,k)->TR[k,j] replicated
    inith_d= dt("inith_d", [81, 128], F32, kind="ExternalInput")    # identity (i==k) replicated cols
    mpid_d = dt("mpid_d", [128, 81], F32, kind="ExternalInput")     # maxplus identity all rows
    mpad_d = dt("mpad_d", [128, 81], F32, kind="ExternalInput")     # maxplus identity where c>c_b else 0
    hid_d  = dt("hid_d", [81, 128], F32, kind="ExternalInput")      # identity blocks where c>c_b else 0
    selm_d = dt("selm_d", [81, S * 128], F32, kind="ExternalInput") # NLL snapshot sel (s,col)
    selm2_d= dt("selm2_d", [128, S * 81], F32, kind="ExternalInput")# vit snapshot sel (s,jk)
    bpm_d  = dt("bpm_d", [128, S * 9], F32, kind="ExternalInput")   # bp doctor mask (s,j)
    bpid_d = dt("bpid_d", [128, S * 9], F32, kind="ExternalInput")  # j*(1-bpm)
    outm_d = dt("outm_d", [128, S], F32, kind="ExternalInput")      # output valid mask
    outneg_d=dt("outneg_d",[128, S], F32, kind="ExternalInput")     # -100*(1-outm)
    emitoh_d=dt("emitoh_d",[128, S * 9], F32, kind="ExternalInput")
    transoh_d=dt("transoh_d",[128, S * 81], F32, kind="ExternalInput")
    corrb_d= dt("corrb_d", [BL, 1], F32, kind="ExternalInput")      # KAPPA*(len-1)
    io9_d  = dt("io9_d", [128, 9], F32, kind="ExternalInput")
    kapb_d = dt("kapb_d", [128, 1], F32, kind="ExternalInput")
    iombig_d = dt("iombig_d", [128, 9], F32, kind="ExternalInput")
    brep_d = dt("brep_d", [128, S * 9], F32, kind="ExternalInput")       # 0..8
    ioj81_d= dt("ioj81_d", [BL, 81], F32, kind="ExternalInput")     # value j at (i,j)
    ioi81_d= dt("ioi81_d", [BL, 81], F32, kind="ExternalInput")     # value i at (i,j)
    idn_d  = dt("idn_d", [128, 128], F32, kind="ExternalInput")
    sumpat_d=dt("sumpat_d",[72, 8], F32, kind="ExternalInput")      # (b,k)->b
    sumpat2_d=dt("sumpat2_d",[128, 8], F32, kind="ExternalInput")   # row (b,c)->b
    ones8_d= dt("ones8_d", [8, 1], F32, kind="ExternalInput")
    # ---- outputs ----
    scr_hsel = dt("scr_hsel", [128, 81], F32)
    scr_gv   = dt("scr_gv", [128, 81], F32)
    scr_v8   = dt("scr_v8", [8, 9], F32)
    scr_vst  = dt("scr_vst", [8, 144], F32)
    scr_ech  = dt("scr_ech", [8, 16], F32)
    loss_o = dt("loss_o", [1, 1], F32, kind="ExternalOutput")
    tags_o = dt("tags_o", [128, S], I32, kind="ExternalOutput")
    if debug:
        dbg = {n: dt(n, shp, F32, kind="ExternalOutput") for n, shp in [
            ("d_lnat", [128, S*9]), ("d_elt", [81, S*128]), ("d_hsel", [128, 81]),
            ("d_gvsel", [128, 81]), ("d_v72", [72, 1]), ("d_den8", [8, 1]),
            ("d_emitc", [128, 1]), ("d_transc", [128, 1]), ("d_vstT", [8, C*9]),
            ("d_w8", [8, 9]), ("d_ltag", [8, 1]), ("d_ech", [8, C]),
            ("d_bpv", [128, S*9]), ("d_vsall", [128, 33*9]), ("d_gvsnap", [128, S*81])]}

    with tile.TileContext(nc) as tc:
      with tc.tile_pool(name="consts", bufs=1) as cp, \
           tc.tile_pool(name="big", bufs=1) as bp_, \
           tc.tile_pool(name="work", bufs=1) as wp, \
           tc.tile_pool(name="ps1", bufs=2, space="PSUM") as ps1, \
           tc.tile_pool(name="ps2", bufs=2, space="PSUM") as ps2:

        # ---------------- load constants ----------------
        eblk = cp.tile([81, 81], F32);   nc.sync.dma_start(eblk[:, :], eblk_d[:, :])
        t81  = cp.tile([128, 81], F32);  nc.sync.dma_start(t81[:, :], t81_d[:, :])
        inith= cp.tile([81, 128], F32);  nc.sync.dma_start(inith[:, :], inith_d[:, :])
        mpid = cp.tile([128, 81], F32);  nc.sync.dma_start(mpid[:, :], mpid_d[:, :])
        mpad = cp.tile([128, 81], F32);  nc.sync.dma_start(mpad[:, :], mpad_d[:, :])
        hidb = cp.tile([81, 128], F32);  nc.sync.dma_start(hidb[:, :], hid_d[:, :])
        io9  = cp.tile([128, 9], F32);   nc.sync.dma_start(io9[:, :], io9_d[:, :])
        kapb = cp.tile([128, 1], F32);   nc.sync.dma_start(kapb[:, :], kapb_d[:, :])
        iombig = cp.tile([128, 9], F32); nc.sync.dma_start(iombig[:, :], iombig_d[:, :])
        bigt = cp.tile([128, 1], F32); nc.gpsimd.memset(bigt[:, :], 1e6)
        brep = wp.tile([128, S * 9], F32); nc.sync.dma_start(brep[:, :], brep_d[:, :])
        ioj81= cp.tile([BL, 81], F32);   nc.sync.dma_start(ioj81[:, :], ioj81_d[:, :])
        ioi81= cp.tile([BL, 81], F32);   nc.sync.dma_start(ioi81[:, :], ioi81_d[:, :])
        idn  = cp.tile([128, 128], F32); nc.sync.dma_start(idn[:, :], idn_d[:, :])
        wrh  = cp.tile([128, KH * 81], BF16)
        wrl  = cp.tile([128, KH * 81], BF16)
        for k in range(KH):
            nc.sync.dma_start(wrh[:, k*81:(k+1)*81], wrep_hi[k, :, :])
            nc.sync.dma_start(wrl[:, k*81:(k+1)*81], wrep_lo[k, :, :])
        selm = bp_.tile([81, S * 128], F32); nc.sync.dma_start(selm[:, :], selm_d[:, :])
        selm2= bp_.tile([128, S * 81], F32); nc.sync.dma_start(selm2[:, :], selm2_d[:, :])
        bpm  = wp.tile([128, S * 9], F32);  nc.sync.dma_start(bpm[:, :], bpm_d[:, :])
        bpid = wp.tile([128, S * 9], F32);  nc.sync.dma_start(bpid[:, :], bpid_d[:, :])
        outm = wp.tile([128, S], F32);      nc.sync.dma_start(outm[:, :], outm_d[:, :])
        outneg=wp.tile([128, S], F32);      nc.sync.dma_start(outneg[:, :], outneg_d[:, :])
        emitoh=wp.tile([128, S * 9], F32);  nc.sync.dma_start(emitoh[:, :], emitoh_d[:, :])
        transoh=bp_.tile([128, S * 81], F32); nc.sync.dma_start(transoh[:, :], transoh_d[:, :])
        corrb= cp.tile([BL, 1], F32);    nc.sync.dma_start(corrb[:, :], corrb_d[:, :])
        sumpat=cp.tile([72, 8], F32);    nc.sync.dma_start(sumpat[:, :], sumpat_d[:, :])
        sumpat2=cp.tile([128, 8], F32);  nc.sync.dma_start(sumpat2[:, :], sumpat2_d[:, :])
        ones8= cp.tile([8, 1], F32);     nc.sync.dma_start(ones8[:, :], ones8_d[:, :])

        # ---------------- P0: hidden transposes + classifier ----------------
        SB = 8  # s'-block
        ELT  = bp_.tile([81, S * 128], F32)
        lnat = wp.tile([128, S * 9], F32)
        lnat31 = wp.tile([128, 16], F32)
        nc.gpsimd.memset(lnat31[:, :], 0.0)
        nc.gpsimd.memset(ELT[:, 31 * 128:32 * 128], 0.0)
        psL_all = {}
        for blk in range(S // SB):
            hT = bp_.tile([128, 2 * KH * SB * 128], BF16, tag="hT")
            for di, hsrc in enumerate((hhi, hlo)):
                for k in range(KH):
                    for si in range(SB):
                        sp = blk * SB + si
                        src_ap = bass.AP(hsrc, sp * HD + k * 128,
                                         [[S * HD, 128], [1, 128]])
                        dst_off = ((di * KH + k) * SB + si) * 128
                        nc.sync.dma_start_transpose(
                            bass.AP(hT.tensor, hT.offset + dst_off, [list(hT.ap[0]), [1, 128]]),
                            src_ap)
            for si in range(SB):
                sp = blk * SB + si
                psL = ps1.tile([81, 128], F32, tag="psL")
                first = True
                for pi in range(3):
                    hsel = (0, 0, 1)[pi]
                    wsel = (wrh, wrl, wrh)[pi]
                    for k in range(KH):
                        rhs_off = ((hsel * KH + k) * SB + si) * 128
                        nc.tensor.matmul(
                            psL[:, :],
                            wsel[:, k*81:(k+1)*81],
                            bass.AP(hT.tensor, hT.offset + rhs_off, [list(hT.ap[0]), [1, 128]]),
                            start=first, stop=(pi == 2 and k == KH - 1))
                        first = False
                if sp >= 1:
                    nc.scalar.activation(ELT[:, (sp-1)*128:sp*128], psL[:, :], ACT.Exp,
                                         bias=kapb[0:81, :], scale=1.0)
                else:
                    inv = V(psL, [[16, 8], [1, 15]], off=1)
                    outv = V(ELT, [[16, 8], [1, 15]], off=31 * 128)
                    nc.scalar.activation(outv, inv, ACT.Exp, bias=kapb[0:81, :], scale=1.0)
                ltmp = wp.tile([16, 128], F32, tag="ltmp")
                nc.vector.tensor_copy(ltmp[0:9, :], psL[0:9, :])
                psT = ps2.tile([128, 16], F32, tag="psT")
                nc.tensor.matmul(psT[:, 0:9], ltmp[0:9, :], idn[0:9, 0:9], start=True, stop=True)
                nc.vector.tensor_copy(lnat[:, sp*9:(sp+1)*9], psT[:, 0:9])
        nc.vector.tensor_tensor(lnat[:, :], lnat[:, :], brep[:, :], op=OP.add)
        # lnat31: row (b,c) <- lnat slice0 row (b,c+1)  (SBUF->SBUF DMA), phantom rows stay 0
        for b in range(BL):
            nc.sync.dma_start(
                bass.AP(lnat31.tensor, lnat31.offset + b * 256, [[16, 15], [1, 9]]),
                bass.AP(lnat.tensor, lnat.offset + b * 4608 + S * 9, [[S * 9, 15], [1, 9]]))
        # v72 init: exp(logits[b,0,:]) ; w8 init: logits[b,0,:]  (rows b*16, slice 0)
        l0 = wp.tile([8, 9], F32)
        nc.sync.dma_start(
            bass.AP(l0.tensor, l0.offset, [[9, 8], [1, 9]]),
            bass.AP(lnat.tensor, lnat.offset, [[16 * S * 9, 8], [1, 9]]))
        v8 = wp.tile([8, 9], F32)
        zb = cp.tile([8, 1], F32); nc.gpsimd.memset(zb[:, :], 0.0)
        nc.scalar.activation(v8[:, :], l0[:, :], ACT.Exp, bias=zb[:, :], scale=1.0)
        v72 = wp.tile([72, 16], F32)
        nc.sync.dma_start(scr_v8[:, :], v8[:, :])
        nc.sync.dma_start(
            bass.AP(v72.tensor, v72.offset, [[16, 72], [1, 1]]),
            bass.AP(scr_v8, 0, [[1, 72], [1, 1]]))
        w8 = wp.tile([8, 9], F32)
        nc.vector.tensor_copy(w8[:, :], l0[:, :])

        # ---------------- P1: matrix-state scans ----------------
        HSNAP = bp_.tile([81, S * 128], F32)
        GVSNAP = bp_.tile([128, S * 81], F32)
        for s in range(S):
            # NLL: H_new = (H @ EHAT) * ELT[s]
            prev = inith[:, :] if s == 0 else HSNAP[:, (s-1)*128:s*128]
            psS = ps2.tile([81, 128], F32, tag="psS")
            nc.tensor.matmul(psS[:, :], eblk[:, :], prev, start=True, stop=True)
            nc.vector.tensor_tensor(HSNAP[:, s*128:(s+1)*128], psS[:, :],
                                    ELT[:, s*128:(s+1)*128], op=OP.mult)
            # Viterbi: GV_new[i,j] = max_k(GV[i,k] + TR[k,j] + logit[j])
            gprev = mpid if s == 0 else None
            psG = ps1.tile([128, 1024], F32, tag="psL")
            if s <= 30:
                lsl = V(lnat, [[0, 9], [1, 9], [0, 9]], off=(s+1)*9)
            else:
                lsl = V(lnat31, [[0, 9], [1, 9], [0, 9]])
            for base, ilo, ni in ((0, 0, 6), (512, 6, 3)):
                pv = bass.AP(psG.tensor, psG.offset + base, [list(psG.ap[0]), [1, ni * 81]])
                if s == 0:
                    gv = V(mpid, [[9, ni], [0, 9], [1, 9]], off=ilo * 9)
                else:
                    gv = V(GVSNAP, [[9, ni], [0, 9], [1, 9]], off=(s-1)*81 + ilo * 9)
                tv = V(t81, [[0, ni], [9, 9], [1, 9]])
                lv = bass.AP(lsl.tensor, lsl.offset, [list(lsl.ap[0]), [0, ni], [1, 9], [0, 9]])
                nc.tensor.matmul(pv, idn[:, :], gv, start=True, stop=False)
                nc.tensor.matmul(pv, idn[:, :], tv, start=False, stop=False)
                nc.tensor.matmul(pv, idn[:, :], lv, start=False, stop=True)
                psv = bass.AP(psG.tensor, psG.offset + base, [list(psG.ap[0]), [81, ni], [9, 9], [1, 9]])
                nc.vector.tensor_reduce(GVSNAP[:, s*81 + ilo*9: s*81 + (ilo+ni)*9],
                                        psv, axis=AX.X, op=OP.max)

        # ---------------- P2: snapshot selection + combines ----------------
        # NLL: Hsel = sum_s HSNAP*selm + HID
        nc.vector.tensor_tensor(
            V(HSNAP, [[1, 128], [128, S]]),
            V(HSNAP, [[1, 128], [128, S]]),
            V(selm, [[1, 128], [128, S]]), op=OP.mult)
        hsel = wp.tile([81, 128], F32)
        nc.vector.tensor_reduce(hsel[:, :], V(HSNAP, [[1, 128], [128, S]]), axis=AX.X, op=OP.add)
        nc.vector.tensor_tensor(hsel[:, :], hsel[:, :], hidb[:, :], op=OP.add)
        # vit: GVsel = sum_s GVSNAP*selm2 + MPAD
        nc.vector.tensor_tensor(
            V(GVSNAP, [[1, 81], [81, S]]),
            V(GVSNAP, [[1, 81], [81, S]]),
            V(selm2, [[1, 81], [81, S]]), op=OP.mult)
        gvsel = wp.tile([128, 81], F32)
        nc.vector.tensor_reduce(gvsel[:, :], V(GVSNAP, [[1, 81], [81, S]]), axis=AX.X, op=OP.add)
        nc.vector.tensor_tensor(gvsel[:, :], gvsel[:, :], mpad[:, :], op=OP.add)

        # NLL combine: BLK assembly (unfold DMAs) + 16 blockdiag matmuls
        BLK = wp.tile([72, C * 72], F32)
        nc.gpsimd.memset(BLK[:, :], 0.0)
        psHT = ps2.tile([128, 128], F32, tag="psS")
        nc.tensor.matmul(psHT[:, 0:81], hsel[:, :], idn[0:81, 0:81], start=True, stop=True)
        hselT = wp.tile([128, 81], F32)
        nc.vector.tensor_copy(hselT[:, :], psHT[:, 0:81])
        nc.sync.dma_start(scr_hsel[:, :], hselT[:, :])
        for b in range(BL):
            # DRAM scr_hsel[(b,c), (k,j)] -> BLK[part b*9+k, c*72+b*9+j]  (k,c,j order)
            nc.sync.dma_start(
                bass.AP(BLK.tensor, BLK.offset + (b*9) * (C*72) + b*9,
                        [[C*72, 9], [72, C], [1, 9]]),
                bass.AP(scr_hsel, b*16*81, [[9, 9], [81, C], [1, 9]]))
        for c in range(C):
            psV = ps2.tile([72, 8], F32, tag="psT")
            nc.tensor.matmul(psV[:, 0:1], BLK[:, c*72:(c+1)*72], v72[:, 0:1], start=True, stop=True)
            nc.vector.tensor_copy(v72[:, 0:1], psV[:, 0:1])
        # denominator: per-b sum over k, ln, + corrb
        psD = ps2.tile([8, 8], F32, tag="psT")
        nc.tensor.matmul(psD[:, 0:1], sumpat[:, :], v72[:, 0:1], start=True, stop=True)
        den8 = wp.tile([8, 1], F32)
        nc.scalar.activation(den8[:, :], psD[:, 0:1], ACT.Ln, bias=zb[:, :], scale=LNSCALE)
        nc.vector.tensor_tensor(den8[:, :], den8[:, :], corrb[:, :], op=OP.add)
        # numerator: emit + trans via STT accum
        junkE = wp.tile([128, S * 9], F32, tag="junkE")
        emitc = wp.tile([128, 1], F32)
        nc.vector.scalar_tensor_tensor(junkE[:, :], lnat[:, :], 1.0, emitoh[:, :],
                                       op0=OP.mult, op1=OP.mult, accum_out=emitc[:, :])
        transc = wp.tile([128, 1], F32)
        trjunk = bp_.tile([128, S * 81], F32, tag="hT")
        nc.vector.scalar_tensor_tensor(trjunk[:, :],
                                       V(t81, [[0, S], [1, 81]]), 1.0, transoh[:, :],
                                       op0=OP.mult, op1=OP.mult, accum_out=transc[:, :])
        numc = wp.tile([128, 1], F32)
        nc.vector.tensor_tensor(numc[:, :], emitc[:, :], transc[:, :], op=OP.add)
        psN = ps2.tile([8, 8], F32, tag="psT")
        nc.tensor.matmul(psN[:, 0:1], sumpat2[:, :], numc[:, :], start=True, stop=True)
        num8 = wp.tile([8, 1], F32)
        nc.vector.tensor_copy(num8[:, :], psN[:, 0:1])
        lvec = wp.tile([8, 1], F32)
        nc.vector.tensor_tensor(lvec[:, :], den8[:, :], num8[:, :], op=OP.subtract)
        psL2 = ps2.tile([1, 8], F32, tag="psT")
        nc.tensor.matmul(psL2[:, 0:1], ones8[:, :], lvec[:, :], start=True, stop=True)
        losst = wp.tile([1, 1], F32)
        nc.vector.tensor_copy(losst[:, :], psL2[:, 0:1])
        nc.sync.dma_start(loss_o[:, :], losst[:, :])

        # vit combine: transpose gvsel -> gvsel_T [8, C*81] (per-b DMA), then 16 steps
        gvselT = wp.tile([8, C * 81], F32)
        nc.sync.dma_start(scr_gv[:, :], gvsel[:, :])
        nc.sync.dma_start(
            bass.AP(gvselT.tensor, gvselT.offset, [[C*81, 8], [1, C*81]]),
            bass.AP(scr_gv, 0, [[C*81, 8], [1, C*81]]))
        vstartT = wp.tile([8, C * 9], F32)
        wtmp = wp.tile([8, 81], F32, tag="wtmp")
        for c in range(C):
            nc.vector.tensor_copy(vstartT[:, c*9:(c+1)*9], w8[:, :])
            nc.vector.tensor_tensor(
                wtmp[:, :],
                gvselT[:, c*81:(c+1)*81],
                V(w8, [[1, 9], [0, 9]]), op=OP.add)      # (i,j): + w[i]
            nc.vector.tensor_reduce(w8[:, :], V(wtmp, [[1, 9], [9, 9]]), axis=AX.X, op=OP.max)
        # last_tag
        wmax = wp.tile([8, 1], F32)
        nc.vector.tensor_reduce(wmax[:, :], w8[:, :], axis=AX.X, op=OP.max)
        junk9 = wp.tile([8, 9], F32, tag="junk9")
        ltag = wp.tile([8, 1], F32)
        nc.vector.scalar_tensor_tensor(junk9[:, :], w8[:, :], wmax[:, :], iombig[0:8, :],
                                       op0=OP.is_equal, op1=OP.mult)
        nc.vector.tensor_reduce(ltag[:, :], junk9[:, :], axis=AX.X, op=OP.min)
        nc.vector.scalar_tensor_tensor(ltag[:, :], ltag[:, :], bigt[0:8, :], ltag[:, :],
                                       op0=OP.add, op1=OP.bypass)
        # E-chain: AVB = gvselT + vstartT[i] + 1e9 ; E[c-1] = argmax_i AVB[c][i, E[c]]
        avbT = wp.tile([8, C * 81], F32)
        nc.vector.tensor_tensor(avbT[:, :], gvselT[:, :],
                                V(vstartT, [[9, C], [1, 9], [0, 9]]), op=OP.add)
        ech = wp.tile([8, C], F32)
        nc.vector.tensor_copy(ech[:, C-1:C], ltag[:, :])
        selv = wp.tile([8, 81], F32, tag="selv")
        colv = wp.tile([8, 9], F32, tag="colv")
        mx1 = wp.tile([8, 1], F32, tag="mx1")
        for c in range(C - 1, 0, -1):
            nc.vector.scalar_tensor_tensor(selv[:, :], ioj81[:, :], ech[:, c:c+1],
                                           avbT[:, c*81:(c+1)*81],
                                           op0=OP.is_equal, op1=OP.mult)
            nc.vector.tensor_reduce(colv[:, :], V(selv, [[9, 9], [1, 9]]), axis=AX.X, op=OP.add)
            nc.vector.tensor_reduce(mx1[:, :], colv[:, :], axis=AX.X, op=OP.max)
            nc.vector.scalar_tensor_tensor(selv[:, 0:9], colv[:, :], mx1[:, :], iombig[0:8, :],
                                           op0=OP.is_equal, op1=OP.mult)
            nc.vector.tensor_reduce(mx1[:, :], selv[:, 0:9], axis=AX.X, op=OP.min)
            nc.vector.scalar_tensor_tensor(ech[:, c-1:c], mx1[:, :], bigt[0:8, :], mx1[:, :],
                                           op0=OP.add, op1=OP.bypass)
        # VSTART rows: vstartT -> VSALL slice 0 rows (b,c) via per-b DMA
        VSALL = wp.tile([128, 33 * 9], F32)
        nc.sync.dma_start(scr_vst[:, :], vstartT[:, :])
        nc.sync.dma_start(
            bass.AP(VSALL.tensor, VSALL.offset, [[33*9, 128], [1, 9]]),
            bass.AP(scr_vst, 0, [[9, 128], [1, 9]]))
        # ---------------- P3: VSNAP re-run + BP + backtrace ----------------
        for s in range(S):
            psW = ps1.tile([128, 1024], F32, tag="psL")
            vsv = V(VSALL, [[0, 9], [1, 9]], off=s * 9)          # (j*, k)
            vsv = bass.AP(VSALL.tensor, VSALL.offset + s * 9, [list(VSALL.ap[0]), [0, 9], [1, 9]])
            tv2 = bass.AP(t81.tensor, t81.offset, [list(t81.ap[0]), [1, 81]])
            if s <= 30:
                lv2 = V(lnat, [[1, 9], [0, 9]], off=(s+1)*9)     # (j, k*)
            else:
                lv2 = V(lnat31, [[1, 9], [0, 9]])
            nc.tensor.matmul(psW[:, 0:81], idn[:, :], vsv, start=True, stop=False)
            nc.tensor.matmul(psW[:, 0:81], idn[:, :], tv2, start=False, stop=False)
            nc.tensor.matmul(psW[:, 0:81], idn[:, :], lv2, start=False, stop=True)
            psWv = bass.AP(psW.tensor, psW.offset, [list(psW.ap[0]), [9, 9], [1, 9]])
            nc.vector.tensor_reduce(VSALL[:, (s+1)*9:(s+2)*9], psWv, axis=AX.X, op=OP.max)
        # BP vectorized: cand[s,j,k] = VSALL[s,k] + TR[k,j] ; argmax_k
        cand = bp_.tile([128, S * 81], F32, tag="hT")
        nc.vector.tensor_tensor(cand[:, :],
                                V(VSALL, [[9, S], [0, 9], [1, 9]]),
                                V(t81, [[0, S], [9, 9], [1, 9]]), op=OP.add)
        mxsj = wp.tile([128, S * 9], F32, tag="mxsj")
        nc.vector.tensor_reduce(mxsj[:, :], V(cand, [[81, S], [9, 9], [1, 9]]), axis=AX.X, op=OP.max)
        nc.vector.tensor_tensor(cand[:, :],
                                V(cand, [[81, S], [9, 9], [1, 9]]),
                                V(mxsj, [[9, S], [1, 9], [0, 9]]), op=OP.is_equal)
        nc.vector.scalar_tensor_tensor(cand[:, :], cand[:, :], 1.0,
                                       V(iombig, [[0, S * 9], [1, 9]]),
                                       op0=OP.mult, op1=OP.mult)
        bpv = wp.tile([128, S * 9], F32, tag="bpv")
        nc.vector.tensor_reduce(bpv[:, :], V(cand, [[81, S], [9, 9], [1, 9]]), axis=AX.X, op=OP.min)
        # +BIG, doctor
        nc.vector.scalar_tensor_tensor(bpv[:, :], bpv[:, :], bigt[:, :], bpm[:, :],
                                       op0=OP.add, op1=OP.mult)
        nc.vector.tensor_tensor(bpv[:, :], bpv[:, :], bpid[:, :], op=OP.add)
        # backtrace: cur rows from ech via per-b DMA; 32 STT steps
        cur = wp.tile([128, 1], F32)
        nc.sync.dma_start(scr_ech[:, :], ech[:, :])
        nc.sync.dma_start(
            bass.AP(cur.tensor, cur.offset, [[1, 128]]),
            bass.AP(scr_ech, 0, [[1, 128]]))
        tagsb = wp.tile([128, S], F32)
        prevcol = cur
        pc_off = 0
        for s in range(S - 1, -1, -1):
            nc.vector.scalar_tensor_tensor(
                junkE[:, 0:9],
                io9[:, :],
                bass.AP(prevcol.tensor, prevcol.offset + pc_off, [list(prevcol.ap[0]), [1, 1]]),
                bpv[:, s*9:(s+1)*9],
                op0=OP.is_equal, op1=OP.mult, accum_out=tagsb[:, s:s+1])
            prevcol = tagsb; pc_off = s
        # mask + cast + out
        nc.vector.tensor_tensor(tagsb[:, :], tagsb[:, :], outm[:, :], op=OP.mult)
        nc.vector.tensor_tensor(tagsb[:, :], tagsb[:, :], outneg[:, :], op=OP.add)
        tagsi = wp.tile([128, S], I32)
        nc.vector.tensor_copy(tagsi[:, :], tagsb[:, :])
        nc.sync.dma_start(tags_o[:, :], tagsi[:, :])
        if debug:
            nc.sync.dma_start(dbg["d_lnat"][:, :], lnat[:, :])
            nc.sync.dma_start(dbg["d_elt"][:, :], ELT[:, :])
            nc.sync.dma_start(dbg["d_hsel"][:, :], bass.AP(hselT.tensor, hselT.offset, [[81, 128], [1, 81]]))
            nc.sync.dma_start(dbg["d_gvsel"][:, :], gvsel[:, :])
            nc.sync.dma_start(dbg["d_v72"][:, :], v72[:, 0:1])
            nc.sync.dma_start(dbg["d_den8"][:, :], den8[:, :])
            nc.sync.dma_start(dbg["d_emitc"][:, :], emitc[:, :])
            nc.sync.dma_start(dbg["d_transc"][:, :], transc[:, :])
            nc.sync.dma_start(dbg["d_vstT"][:, :], vstartT[:, :])
            nc.sync.dma_start(dbg["d_w8"][:, :], w8[:, :])
            nc.sync.dma_start(dbg["d_ltag"][:, :], ltag[:, :])
            nc.sync.dma_start(dbg["d_ech"][:, :], ech[:, :])
            nc.sync.dma_start(dbg["d_bpv"][:, :], bpv[:, :])
            nc.sync.dma_start(dbg["d_vsall"][:, :], VSALL[:, :])
            nc.sync.dma_start(dbg["d_gvsnap"][:, :], GVSNAP[:, :])

    nc.compile()
    _prog_cache[('p', debug)] = nc
    return nc


def host_prep(hidden, W, b, transitions, labels, attention_mask):
    """Build per-core input maps."""
    hidden = np.asarray(hidden, np.float32)
    W = np.asarray(W, np.float32); bv = np.asarray(b, np.float32)
    TR = np.asarray(transitions, np.float32)
    labels = np.asarray(labels); am = np.asarray(attention_mask)
    lengths = (am != 0).sum(1).astype(np.int64)
    tags = np.where(labels == -100, 0, labels).astype(np.int64)

    hhi = hidden.astype(ml_dtypes.bfloat16)
    hlo = (hidden - hhi.astype(np.float32)).astype(ml_dtypes.bfloat16)
    # classifier weights with bias folded: logits = h @ W.T + b. Fold b via... b is zeros in
    # setup, but keep general: add b later on host? b enters loss/viterbi. Fold into WREP? No h
    # column for bias; append to kappa path is wrong. We add b via ELT/lnat corrections below if
    # nonzero. For now assert-ish: handled by adding b into lnat/ELT host-side is impossible
    # (device computes). Instead fold b by shifting W's product: append bias as extra hidden dim
    # would change shapes; simplest: absorb approximately by noting b==0 in this problem.
    WT = W.T.copy()  # [768, 9]
    wrep = np.zeros((KH, 128, 81), np.float32)
    for k in range(KH):
        blk = WT[k*128:(k+1)*128]                    # [128, 9]
        wrep[k] = np.tile(blk, (1, 9))               # col (i*9+j) -> W[j, h]
    wrep_hi = wrep.astype(ml_dtypes.bfloat16)
    wrep_lo = (wrep - wrep_hi.astype(np.float32)).astype(ml_dtypes.bfloat16)

    EHAT = np.exp(TR).astype(np.float32)
    eblk = np.zeros((81, 81), np.float32)
    for i in range(9):
        eblk[i*9:(i+1)*9, i*9:(i+1)*9] = EHAT        # lhsT[(i,k),(i,j)] = EHAT[k,j]
    t81 = np.tile(TR.T.reshape(1, 81), (128, 1)).astype(np.float32)  # (j,k)->TR[k,j]
    inith = np.tile(np.eye(9, dtype=np.float32).reshape(81, 1), (1, 128))
    mpidv = np.where(np.eye(9, dtype=bool), 0.0, -1e9).astype(np.float32).reshape(81)
    mpid = np.tile(mpidv, (128, 1))
    io9 = np.tile(np.arange(9, dtype=np.float32), (128, 1))
    ioj81 = np.tile(np.tile(np.arange(9, dtype=np.float32), 9), (BL, 1))
    ioi81 = np.tile(np.repeat(np.arange(9, dtype=np.float32), 9), (BL, 1))
    idn = np.eye(128, dtype=np.float32)
    sumpat = np.zeros((72, 8), np.float32)
    for b_ in range(8):
        sumpat[b_*9:(b_+1)*9, b_] = 1.0
    sumpat2 = np.zeros((128, 8), np.float32)
    for b_ in range(8):
        sumpat2[b_*16:(b_+1)*16, b_] = 1.0
    ones8 = np.ones((8, 1), np.float32)
    kapb = np.zeros((128, 1), np.float32) - KAPPA
    kapb[:81, 0] = np.tile(bv, 9) - KAPPA
    brep = np.tile(bv, (128, S)).astype(np.float32)

    in_maps = []
    for core in range(NC):
        bsl = slice(core*BL, (core+1)*BL)
        len8 = lengths[bsl]; tg8 = tags[bsl]
        t_last = len8 - 1
        c_b = (t_last - 1) // S
        s_b = (t_last - 1) % S
        # selm [81, (s,col)] col=b*16+c
        selm = np.zeros((S, 128), np.float32)
        selm2 = np.zeros((128, S), np.float32)
        mpad = np.zeros((128, 81), np.float32)
        hidbv = np.zeros((81, 128), np.float32)
        for b_ in range(BL):
            for c in range(C):
                col = b_*16 + c
                if c < c_b[b_]:
                    selm[S-1, col] = 1.0; selm2[col, S-1] = 1.0
                elif c == c_b[b_]:
                    selm[s_b[b_], col] = 1.0; selm2[col, s_b[b_]] = 1.0
                else:
                    mpad[col] = mpidv
                    hidbv[:, col] = np.eye(9, dtype=np.float32).reshape(81)
        selm_rep = np.tile(selm.reshape(1, S, 128), (81, 1, 1)).reshape(81, S*128)
        selm2_rep = np.repeat(selm2, 81, axis=1).reshape(128, S, 81).reshape(128, S*81)
        # bp doctor: bpm[row,(s,j)] = 1 if t=c*32+s+1 <= t_last
        bpm = np.zeros((128, S, 9), np.float32)
        bpid = np.zeros((128, S, 9), np.float32)
        outmv = np.zeros((128, S), np.float32)
        outneg = np.zeros((128, S), np.float32)
        emitoh = np.zeros((128, S, 9), np.float32)
        transoh = np.zeros((128, S, 81), np.float32)
        for b_ in range(BL):
            for c in range(C):
                row = b_*16 + c
                for s in range(S):
                    t = c*S + s + 1
                    if t <= t_last[b_]:
                        bpm[row, s] = 1.0
                    else:
                        bpid[row, s] = np.arange(9, dtype=np.float32)
                    tt_ = c*S + s     # tag position for output/emit
                    if tt_ <= t_last[b_]:
                        outmv[row, s] = 1.0
                        emitoh[row, s, tg8[b_, tt_]] = 1.0
                    else:
                        outneg[row, s] = -100.0
                    if tt_ + 1 <= t_last[b_]:
                        transoh[row, s, tg8[b_, tt_+1]*9 + tg8[b_, tt_]] = 1.0
        corrb = (KAPPA * (len8 - 1) + 32.0 * np.log(2.0)).astype(np.float32).reshape(8, 1)
        in_maps.append({
            "hhi": hhi[bsl], "hlo": hlo[bsl],
            "wrep_hi": wrep_hi, "wrep_lo": wrep_lo,
            "eblk_d": eblk, "t81_d": t81, "inith_d": inith, "mpid_d": mpid,
            "mpad_d": mpad, "hid_d": hidbv,
            "selm_d": selm_rep, "selm2_d": selm2_rep,
            "bpm_d": bpm.reshape(128, S*9), "bpid_d": bpid.reshape(128, S*9),
            "outm_d": outmv, "outneg_d": outneg,
            "emitoh_d": emitoh.reshape(128, S*9), "transoh_d": transoh.reshape(128, S*81),
            "corrb_d": corrb, "io9_d": io9, "ioj81_d": ioj81, "ioi81_d": ioi81,
            "idn_d": idn, "sumpat_d": sumpat, "sumpat2_d": sumpat2, "ones8_d": ones8,
            "kapb_d": kapb, "brep_d": brep, "iombig_d": io9 - 1e6,
        })
    return in_maps, lengths


def assemble(results, lengths):
    loss = np.float32(0.0)
    pred = np.full((B, T), -100, np.int32)
    for core in range(NC):
        loss += results[core]["loss_o"][0, 0]
        tg = results[core]["tags_o"]          # [128, 32] rows b*16+c
        for b_ in range(BL):
            bb = core*BL + b_
            tags_flat = tg[b_*16:(b_+1)*16].reshape(T)   # t = c*32+s
            ln = int(lengths[bb])
            pred[bb, 1:ln+1] = tags_flat[0:ln]
    return np.float32(loss), pred


_last_exec_ns = None

def kernel(hidden, W, b, transitions, labels, attention_mask, _trace=False):
    global _last_exec_ns
    nc = build_program()
    in_maps, lengths = host_prep(hidden, W, b, transitions, labels, attention_mask)
    res = run_bass_kernel_spmd(nc, in_maps, list(range(NC)), trace=_trace)
    if getattr(res, "exec_time_ns", None) is not None:
        _last_exec_ns = res.exec_time_ns
    return assemble(res.results, lengths)
